# revision 1
# baseline (speedup 1.0000x reference)
"""Deformable 3D conv net on 8 Trainium2 NeuronCores (Bass/Tile), v2.

Sharding: core (b, q) = batch b in {0,1} x D-quarter q in {0..3};
each core computes out[b, :, 12q:12q+12, :, :] from a padded x slab.

Per-core algorithm (exact trilinear, 5-wide window, exact for |off|<=2;
offsets clamped to [-2,2] on device):
  Phase 0 (all slices): PE off-conv -> off[81, n] -> ACT hats
     h_s = relu(1-|off-s|) bf16, s=-2..2, stored to DRAM.
  Phase 1 per slice: per tap k three replicating DMAs build
     zd/zh/zw[125, n] from the hat rows (delta-partition layout);
     zeta_k = zd*zh*zw (2 DVE mults, bf16). Per channel c: xr = 125
     delta-shifted replicas of padded x channel c (one replicating
     DMA); per tap: P = zeta_k * xr_window (DVE or Pool, bf16);
     PE matmul K=128 with stationary w_dc[o,c,k] broadcast over rows
     accumulates out[32, n] in PSUM over all (k, c).
"""

import numpy as np
import ml_dtypes

import concourse.bass as bass
import concourse.bacc as bacc
import concourse.mybir as mybir
from concourse.tile import TileContext
from concourse.bass_utils import run_bass_kernel_spmd

B, C, O, S = 2, 32, 32, 48
KS, KV = 3, 27
PAD = 4
DP = 12                 # output D per core
DPP = DP + 2 * PAD      # 20
HP = WP = S + 2 * PAD   # 56
HWP = HP * WP           # 3136
NPAD = DPP * HWP        # 62720
NDELTA = 125
NN = S * S              # 2304

F32 = mybir.dt.float32
BF16 = mybir.dt.bfloat16
ALU = mybir.AluOpType
ACTF = mybir.ActivationFunctionType

TAP_GROUPS = [list(range(0, 14)), list(range(14, 27))]
NCHUNKS = [(0, 512), (512, 512), (1024, 512), (1536, 512), (2048, 256)]
NS_LOOP = DP  # number of d-slices traced (reduce for simulation tests)
POOL_TAPS = frozenset({2, 7, 11, 16, 21, 25})
LAST_RESULTS = None


# ---------------------------------------------------------------- host prep
def _build_core_inputs(x, w_off, b_off, w_dc, b_dc, b, q):
    xp = np.zeros((C, DPP, HP, WP), np.float32)
    d0 = DP * q - PAD
    lo, hi = max(0, -d0), min(DPP, S - d0)
    xp[:, lo:hi, PAD:PAD + S, PAD:PAD + S] = x[b, :, d0 + lo:d0 + hi]

    # x3[32g+c, d, h, w] = xp[c, d, h, w + (g-1)]  (wrap lands in zero pad)
    x3 = np.zeros((96, DPP, HP, WP), np.float32)
    for g in range(3):
        x3[32 * g:32 * g + 32] = np.roll(xp, -(g - 1), axis=3)
    x3 = x3.reshape(96, NPAD).astype(ml_dtypes.bfloat16)

    x_bf = xp.reshape(C, NPAD).astype(ml_dtypes.bfloat16)

    # w_off9: [9*96, 81]: chunk (kd,kh), rows (kw, c), cols m = 3k + axis
    woff = w_off.reshape(KV, 3, C, KS, KS, KS)
    w_off9 = np.zeros((9, 96, 81), np.float32)
    for kd in range(3):
        for kh in range(3):
            ch = kd * 3 + kh
            for kw in range(3):
                blk = woff[:, :, :, kd, kh, kw]          # (k, ax, c)
                w_off9[ch, 32 * kw:32 * kw + 32, :] = \
                    blk.transpose(2, 0, 1).reshape(C, KV * 3)
    w_off9 = w_off9.astype(ml_dtypes.bfloat16)

    # wdc_rep: [128, KV*C*O]: rows = delta (125 used), free (k, c, o)
    wdcf = w_dc.reshape(O, C, KV)
    wdc = np.zeros((128, KV * C * O), np.float32)
    wdc[:NDELTA, :] = wdcf.transpose(2, 1, 0).reshape(KV * C * O)[None, :]
    wdc = wdc.astype(ml_dtypes.bfloat16)

    # svals[p, j]: bias columns -s for s=-2..2 (for |off - s|)
    sv = np.zeros((81, 5), np.float32)
    sv[:, :] = -np.arange(-2, 3)[None, :]

    return {
        "x3": np.ascontiguousarray(x3),
        "x_bf": np.ascontiguousarray(x_bf),
        "w_off9": np.ascontiguousarray(w_off9.transpose(1, 0, 2).reshape(96, 9 * 81)),
        "wdc_rep": np.ascontiguousarray(wdc),
        "b_off": np.ascontiguousarray(b_off.astype(np.float32).reshape(81, 1)),
        "b_dc": np.ascontiguousarray(b_dc.astype(np.float32).reshape(32, 1)),
        "svals": np.ascontiguousarray(sv),
        "zeros3": np.zeros((3, 3 * HWP), ml_dtypes.bfloat16),
        "zeros_np": np.zeros((1, NPAD), ml_dtypes.bfloat16),
    }


# ---------------------------------------------------------------- device IR
def _win_ap(dram_row_ap, offset, ap_dims):
    a = dram_row_ap.copy()
    a.ap = mybir.VecI64Pair(ap_dims)
    a.offset = offset
    return a


def build_kernel(nc: bass.Bass):
    x3_d = nc.dram_tensor("x3", [96, NPAD], BF16, kind="ExternalInput")
    xbf_d = nc.dram_tensor("x_bf", [C, NPAD], BF16, kind="ExternalInput")
    woff_d = nc.dram_tensor("w_off9", [96, 9 * 81], BF16, kind="ExternalInput")
    wdc_d = nc.dram_tensor("wdc_rep", [128, KV * C * O], BF16,
                           kind="ExternalInput")
    boff_d = nc.dram_tensor("b_off", [81, 1], F32, kind="ExternalInput")
    bdc_d = nc.dram_tensor("b_dc", [32, 1], F32, kind="ExternalInput")
    sv_d = nc.dram_tensor("svals", [81, 5], F32, kind="ExternalInput")
    z3_d = nc.dram_tensor("zeros3", [3, 3 * HWP], BF16, kind="ExternalInput")
    znp_d = nc.dram_tensor("zeros_np", [1, NPAD], BF16, kind="ExternalInput")
    xrep_ds = [nc.dram_tensor(f"xrep_scratch{i}", [8 * NDELTA, NPAD], BF16,
                              kind="Internal") for i in range(4)]
    hats_d = nc.dram_tensor("hats_scratch", [1, NS_LOOP * 5 * 81 * NN], BF16,
                            kind="Internal")
    out_d = nc.dram_tensor("out", [O, NS_LOOP * NN], F32, kind="ExternalOutput")

    with TileContext(nc) as tc:
        with tc.tile_pool(name="fixed", bufs=1) as fixed:
            woff_s = fixed.tile([96, 9 * 81], BF16)
            nc.sync.dma_start(woff_s[:, :], woff_d[:, :])
            boff_s = fixed.tile([81, 1], F32)
            nc.sync.dma_start(boff_s[:, :], boff_d[:, :])
            bdc_s = fixed.tile([32, 1], F32)
            nc.sync.dma_start(bdc_s[:, :], bdc_d[:, :])
            sv_s = fixed.tile([81, 5], F32)
            nc.sync.dma_start(sv_s[:, :], sv_d[:, :])

            # rotating buffers with zero-padded rows 125..127 (set once)
            xrs = [fixed.tile([128, 3, HP, WP], BF16, name=f"xrbuf{i}")
                   for i in range(2)]
            for t in xrs:
                nc.scalar.dma_start(
                    t.rearrange("p a h w -> p (a h w)")[NDELTA:128, :],
                    z3_d[:, :])
            zts = [fixed.tile([128, NN], BF16, name=f"ztbuf{i}")
                   for i in range(14)]
            for t in zts:
                nc.scalar.dma_start(t[NDELTA:128, :], z3_d[:, 0:NN])

            wdcs_fixed = []
            for gi, taps in enumerate(TAP_GROUPS):
                k0 = taps[0]
                wt = fixed.tile([128, len(taps) * C * O], BF16,
                                name=f"wdcg{gi}")
                nc.sync.dma_start(
                    wt[:, :], wdc_d[:, k0 * C * O:(k0 + len(taps)) * C * O])
                wdcs_fixed.append(wt)

            # warm fixed tiles on DVE once so later instructions don't
            # each carry a DMA-sem wait (HW wait-slot limit)
            warm = fixed.tile([1, 8], F32)
            for wsrc in [boff_s, bdc_s, sv_s]:
                nc.vector.tensor_copy(warm[0:1, 0:1], wsrc[0:1, 0:1])

            # ---------------- build xrep: 125 shifted replicas ---------
            # xrep[c*125+p, m] = xp[c][m + shift(p)], zero where OOB
            for c in range(C):
                dst = _win_ap(xrep_ds[c // 8][0:1, :],
                              (c % 8) * NDELTA * NPAD,
                              [(NPAD, NDELTA), (1, NPAD)])
                nc.sync.dma_start(
                    dst, _win_ap(znp_d[0:1, :], 0, [(0, NDELTA), (1, NPAD)]))
            for p in range(NDELTA):
                sh = ((p // 25 - 2) * HWP + ((p // 5) % 5 - 2) * WP
                      + (p % 5 - 2))
                L = NPAD - abs(sh)
                for ci in range(4):
                    dst = _win_ap(xrep_ds[ci][0:1, :],
                                  p * NPAD + max(0, -sh),
                                  [(NDELTA * NPAD, 8), (1, L)])
                    src_ap = _win_ap(xbf_d[0:1, :], ci * 8 * NPAD + max(0, sh),
                                     [(NPAD, 8), (1, L)])
                    nc.sync.dma_start(dst, src_ap)

            # ---------------- phase 0: off-conv + hats, all slices ----
            with tc.tile_pool(name="ph0", bufs=1) as pool, \
                 tc.tile_pool(name="ph0ps", bufs=1, space="PSUM") as psp:
                for ds in range(NS_LOOP):
                    _off_and_hats(nc, pool, psp, ds, x3_d, hats_d,
                                  woff_s, boff_s, sv_s)

            # ---------------- phase 1: main loop per slice ------------
            for ds in range(NS_LOOP):
                with tc.tile_pool(name=f"sl{ds}", bufs=1) as pool, \
                     tc.tile_pool(name=f"ps{ds}", bufs=1, space="PSUM") as psp:
                    _do_slice(nc, pool, psp, ds, xrep_ds, wdcs_fixed, hats_d,
                              out_d, bdc_s, xrs, zts)
    return nc


def _off_and_hats(nc, pool, psp, ds, x3_d, hats_d, woff_s, boff_s, sv_s):
    dpad = ds + PAD
    x3s = pool.tile([96, 3, HP, WP], BF16, name=f"x3s{ds}", tag="x3s")
    nc.scalar.dma_start(
        x3s.rearrange("p a h w -> p (a h w)"),
        x3_d[:, (dpad - 1) * HWP:(dpad + 2) * HWP])
    off = pool.tile([81, S, S], F32, name=f"off{ds}", tag="off")
    for hc in range(5):
        hb, hn = 10 * hc, (8 if hc == 4 else 10)
        ps = psp.tile([81, hn, S], F32, name=f"offps{ds}_{hc}", tag="offps")
        for i in range(9):
            kd, kh = i // 3, i % 3
            rhs = x3s[:, kd, 3 + kh + hb:3 + kh + hb + hn, 4:52]
            nc.tensor.matmul(ps[:], woff_s[:, i * 81:(i + 1) * 81],
                             rhs, start=(i == 0), stop=(i == 8))
        nc.vector.tensor_scalar(off[:, hb:hb + hn, :], ps[:],
                                boff_s[:, :], 2.0, ALU.add, ALU.min)
    nc.vector.tensor_scalar(off[:], off[:], -2.0, None, ALU.max)

    offf = off.rearrange("p h w -> p (h w)")
    hbase = ds * 5 * 81 * NN
    for s in range(5):
        u = pool.tile([81, NN], BF16, name=f"u{ds}_{s}", tag="hatu")
        nc.scalar.activation(u[:], offf, ACTF.Abs,
                             bias=sv_s[:, s:s + 1], scale=1.0)
        h_s = pool.tile([81, NN], BF16, name=f"h{ds}_{s}", tag="hat", bufs=2)
        nc.scalar.activation(h_s[:], u[:], ACTF.Relu, bias=1.0, scale=-1.0)
        nc.sync.dma_start(
            _win_ap(hats_d[0:1, :], hbase + s * 81 * NN,
                    [(NN, 81), (1, NN)]),
            h_s[:, :])


def _do_slice(nc, pool, psp, ds, xrep_ds, wdcs_fixed, hats_d, out_d, bdc_s,
              xrs, zts):
    dpad = ds + PAD
    hbase = ds * 5 * 81 * NN

    accs = [psp.tile([O, nn], F32, name=f"acc{ds}_{ci}", tag=f"acc{ci}")
            for ci, (nb, nn) in enumerate(NCHUNKS)]
    first_mm = [True] * len(NCHUNKS)

    for gi, taps in enumerate(TAP_GROUPS):
        k0 = taps[0]
        wdc_s = wdcs_fixed[gi]

        zetas = {}
        for k in taps:
            zd = pool.tile([NDELTA, NN], BF16, name=f"zd{ds}_{k}",
                           tag="zd", bufs=2)
            zh = pool.tile([NDELTA, NN], BF16, name=f"zh{ds}_{k}",
                           tag="zh", bufs=2)
            zw = pool.tile([NDELTA, NN], BF16, name=f"zw{ds}_{k}",
                           tag="zw", bufs=2)
            # z*[p] = hat_{s*(p)}[row 3k+ax]; p = sd*25 + sh*5 + sw
            for sd in range(5):
                nc.scalar.dma_start(
                    zd[25 * sd:25 * sd + 25, :],
                    _win_ap(hats_d[0:1, :],
                            hbase + sd * 81 * NN + (3 * k + 0) * NN,
                            [(0, 25), (1, NN)]))
            for sd in range(5):
                nc.scalar.dma_start(
                    zh[25 * sd:25 * sd + 25, :],
                    _win_ap(hats_d[0:1, :], hbase + (3 * k + 1) * NN,
                            [(81 * NN, 5), (0, 5), (1, NN)]))
            nc.scalar.dma_start(
                zw[:, :],
                _win_ap(hats_d[0:1, :], hbase + (3 * k + 2) * NN,
                        [(0, 25), (81 * NN, 5), (1, NN)]))
            z = zts[k - k0]
            nc.vector.tensor_tensor(zd[:], zd[:], zh[:], ALU.mult)
            nc.vector.tensor_tensor(z[0:NDELTA, :], zd[:], zw[:], ALU.mult)
            zetas[k] = z

        last = (gi == len(TAP_GROUPS) - 1)
        for c in range(C):
            xr = xrs[c % 2]
            xrf = xr.rearrange("p a h w -> p (a h w)")
            nc.sync.dma_start(
                xrf[0:NDELTA, :],
                _win_ap(xrep_ds[c // 8][0:1, :],
                        (c % 8) * NDELTA * NPAD + (dpad - 1) * HWP,
                        [(NPAD, NDELTA), (1, 3 * HWP)]))
            for k in taps:
                kd, kh, kw = k // 9, (k // 3) % 3, k % 3
                win = xr[:, kd, 3 + kh:3 + kh + S, 3 + kw:3 + kw + S]
                p = pool.tile([128, S, S], BF16,
                              name=f"p{ds}_{gi}_{c}_{k}", tag="ptile",
                              bufs=3)
                eng = nc.gpsimd if k in POOL_TAPS else nc.vector
                eng.tensor_tensor(
                    p[:], zetas[k][:, :].rearrange("p (h w) -> p h w", h=S),
                    win, ALU.mult)
                pf = p.rearrange("p h w -> p (h w)")
                wsl = wdc_s[:, ((k - k0) * C + c) * O:
                            ((k - k0) * C + c + 1) * O]
                fin = last and (c == C - 1) and (k == taps[-1])
                for ci, (nb, nn) in enumerate(NCHUNKS):
                    nc.tensor.matmul(accs[ci][:], wsl, pf[:, nb:nb + nn],
                                     start=first_mm[ci], stop=fin)
                    first_mm[ci] = False

    for ci, (nb, nn) in enumerate(NCHUNKS):
        outp = pool.tile([O, nn], F32, name=f"outp{ds}_{ci}", tag="outp",
                         bufs=2)
        nc.vector.tensor_scalar(outp[:, :], accs[ci][:], bdc_s[:, :],
                                None, ALU.add)
        nc.sync.dma_start(out_d[:, ds * NN + nb:ds * NN + nb + nn],
                          outp[:, :])


# ---------------------------------------------------------------- entry
def kernel(x, w_off, b_off, w_dc, b_dc):
    x = np.asarray(x, np.float32)
    w_off = np.asarray(w_off, np.float32)
    b_off = np.asarray(b_off, np.float32)
    w_dc = np.asarray(w_dc, np.float32)
    b_dc = np.asarray(b_dc, np.float32)

    in_maps = [_build_core_inputs(x, w_off, b_off, w_dc, b_dc,
                                  core // 4, core % 4) for core in range(8)]

    nc = bacc.Bacc("TRN2", target_bir_lowering=False, debug=False,
                   enable_asserts=False, num_devices=8)
    build_kernel(nc)
    if not nc.is_finalized():
        nc.finalize()

    global LAST_RESULTS
    LAST_RESULTS = run_bass_kernel_spmd(nc, in_maps, list(range(8)))
    res = LAST_RESULTS.results

    out = np.zeros((B, O, S, S, S), np.float32)
    for core in range(8):
        b, q = core // 4, core % 4
        out[b, :, DP * q:DP * q + NS_LOOP] = \
            res[core]["out"].reshape(O, NS_LOOP, S, S).astype(np.float32)
    return out

